# revision 11
# baseline (speedup 1.0000x reference)
"""BinaryTreeLSTMCell fused kernel for 8 TRN2 NeuronCores.

Strategy: data-parallel over the batch (8192 rows -> 1024 rows/core).
Per core, compute gates^T = W @ [x|h_left|h_right]^T (K=3072 contraction,
5120 gate rows) with fp32r matmuls (1 cycle/row at free>=256, ~tf32
precision), bias+sigmoid/tanh fused on ScalarE straight out of PSUM,
LSTM cell elementwise on VectorE, all in a gate-major (transposed)
layout so the contraction dim sits on SBUF partitions for both matmul
operands. Host pre-transposes the activations and pre-tiles W so every
DMA is wide and contiguous.
"""

import numpy as np

import concourse.bacc as bacc
import concourse.mybir as mybir
import concourse.tile as tile
from concourse.bass_utils import run_bass_kernel_spmd

F32 = mybir.dt.float32
F32R = mybir.dt.float32r
AF = mybir.ActivationFunctionType

N_CORES = 8
B = 8192
IN_SIZE = 1024
HID = 1024
COMB = IN_SIZE + 2 * HID          # 3072 contraction dim
NGATES = 5 * HID                  # 5120 stacked gate rows
BS = B // N_CORES                 # 1024 batch rows per core
KT = COMB // 128                  # 24 k-tiles
NT = NGATES // 128                # 40 gate tiles
JT = HID // 128                   # 8 h-slices
BB = BS // 512                    # 2 moving blocks of 512

_NC = {}


def _build(repeat=1):
    if repeat in _NC:
        return _NC[repeat]

    nc = bacc.Bacc("TRN2", target_bir_lowering=False, debug=False)

    combT = nc.dram_tensor("combT", [COMB, BS], F32R, kind="ExternalInput").ap()
    wbig = nc.dram_tensor("wbig", [NT, 128, COMB], F32R, kind="ExternalInput").ap()
    bias = nc.dram_tensor("bias", [128, NT], F32, kind="ExternalInput").ap()
    clT = nc.dram_tensor("clT", [HID, BS], F32, kind="ExternalInput").ap()
    crT = nc.dram_tensor("crT", [HID, BS], F32, kind="ExternalInput").ap()
    hT = nc.dram_tensor("hT", [HID, BS], F32, kind="ExternalOutput").ap()
    cT = nc.dram_tensor("cT", [HID, BS], F32, kind="ExternalOutput").ap()

    with tile.TileContext(nc) as tc:
        with (
            tc.tile_pool(name="const", bufs=1) as const_pool,
            tc.tile_pool(name="comb", bufs=1) as comb_pool,
            tc.tile_pool(name="w", bufs=3) as w_pool,
            tc.tile_pool(name="gates", bufs=2) as gate_pool,
            tc.tile_pool(name="cc", bufs=2) as cc_pool,
            tc.tile_pool(name="ew", bufs=1) as ew_pool,
            tc.tile_pool(name="psum", bufs=8, space="PSUM") as psum_pool,
        ):
            bias_sb = const_pool.tile([128, NT], F32, tag="bias")
            nc.sync.dma_start(bias_sb[:], bias[:])

            # Load the first batch-half of every k-tile first so the first
            # accumulation groups (which read only columns [0,512)) can
            # start while the second half streams in.
            comb_sb = []
            for k in range(KT):
                ct = comb_pool.tile([128, BS], F32R, tag=f"comb{k}")
                comb_sb.append(ct)
            for bb in range(BB):
                for k in range(KT):
                    nc.sync.dma_start(
                        comb_sb[k][:, bb * 512:(bb + 1) * 512],
                        combT[k * 128:(k + 1) * 128, bb * 512:(bb + 1) * 512],
                    )

            for j in [jj for _ in range(repeat) for jj in range(JT)]:
                cl_t = cc_pool.tile([128, BS], F32, tag="cl")
                cr_t = cc_pool.tile([128, BS], F32, tag="cr")
                nc.sync.dma_start(cl_t[:], clT[j * 128:(j + 1) * 128, :])
                nc.sync.dma_start(cr_t[:], crT[j * 128:(j + 1) * 128, :])

                gates = []
                for g in range(5):
                    n = g * JT + j
                    wt = w_pool.tile([128, COMB], F32R, tag="wt")
                    nc.sync.dma_start(wt[:], wbig[n])
                    gt = gate_pool.tile([128, BS], F32, tag=f"g{g}")
                    func = AF.Tanh if g == 4 else AF.Sigmoid
                    for bb in range(BB):
                        ps = psum_pool.tile([128, 512], F32, tag="ps")
                        for k in range(KT):
                            nc.tensor.matmul(
                                ps[:],
                                wt[:, k * 128:(k + 1) * 128],
                                comb_sb[k][:, bb * 512:(bb + 1) * 512],
                                start=(k == 0),
                                stop=(k == KT - 1),
                            )
                        nc.scalar.activation(
                            gt[:, bb * 512:(bb + 1) * 512],
                            ps[:],
                            func,
                            bias=bias_sb[:, n:n + 1],
                        )
                    gates.append(gt)

                i_t, fl_t, fr_t, o_t, u_t = gates
                c_t = ew_pool.tile([128, BS], F32, tag="c")
                h_t = ew_pool.tile([128, BS], F32, tag="h")
                tmp = ew_pool.tile([128, BS], F32, tag="tmp")
                nc.vector.tensor_mul(c_t[:], i_t[:], u_t[:])
                nc.vector.tensor_mul(tmp[:], fl_t[:], cl_t[:])
                nc.vector.tensor_add(c_t[:], c_t[:], tmp[:])
                nc.vector.tensor_mul(tmp[:], fr_t[:], cr_t[:])
                nc.vector.tensor_add(c_t[:], c_t[:], tmp[:])
                nc.scalar.activation(h_t[:], c_t[:], AF.Tanh)
                nc.vector.tensor_mul(h_t[:], o_t[:], h_t[:])
                nc.sync.dma_start(cT[j * 128:(j + 1) * 128, :], c_t[:])
                nc.sync.dma_start(hT[j * 128:(j + 1) * 128, :], h_t[:])

    nc.compile()
    _NC[repeat] = nc
    return nc


def make_in_maps(x, h_left, c_left, h_right, c_right, W, b):
    x, h_left, c_left, h_right, c_right, W, b = (
        np.asarray(a, dtype=np.float32)
        for a in (x, h_left, c_left, h_right, c_right, W, b)
    )
    comb = np.concatenate([x, h_left, h_right], axis=1)
    # wbig[n, p, k*128+m] = W[n*128+m, k*128+p]: per gate-tile n, a
    # (128 kpart, 24*128) block whose partition lines are contiguous.
    wbig = np.ascontiguousarray(
        W.reshape(NT, 128, KT, 128).transpose(0, 3, 2, 1).reshape(NT, 128, COMB)
    )
    bias_arr = np.ascontiguousarray(b.reshape(NT, 128).T)
    in_maps = []
    for i in range(N_CORES):
        sl = slice(i * BS, (i + 1) * BS)
        in_maps.append({
            "combT": np.ascontiguousarray(comb[sl].T),
            "wbig": wbig,
            "bias": bias_arr,
            "clT": np.ascontiguousarray(c_left[sl].T),
            "crT": np.ascontiguousarray(c_right[sl].T),
        })
    return in_maps


def kernel(x, h_left, c_left, h_right, c_right, W, b):
    nc = _build()
    in_maps = make_in_maps(x, h_left, c_left, h_right, c_right, W, b)
    res = run_bass_kernel_spmd(nc, in_maps, list(range(N_CORES)))
    h = np.concatenate([res.results[i]["hT"].T for i in range(N_CORES)], axis=0)
    c = np.concatenate([res.results[i]["cT"].T for i in range(N_CORES)], axis=0)
    return h, c
